# revision 1
# baseline (speedup 1.0000x reference)
"""Trainium2 Bass kernel for channel ("transposed") attention:
  qkv = conv3x3(conv1x1(x)); per-head L2-normalized channel attention; 1x1 proj.

Sharding: pure data-parallel — batch 8 across 8 NeuronCores (one image each).
Per-core pipeline (all matmuls bf16 with f32 PSUM accumulation):
  A: y1p = w1 @ xp (host-padded input, 130x130) -> DRAM (bf16)
  B: conv3x3 as 9 shifted matmuls x 5 k-tiles accumulated in PSUM;
     v kept SBUF-resident, q/k streamed to DRAM + squared-norm reduce;
     per-n-tile qk DMA-transposes and per-head logit matmuls interleaved
     (one tile lagged) so the PE stream never breaks
  C: norm/scale softmax on the tiny [48,48] logits; FW = wproj @ blockdiag(A)
  D: out = FW @ v -> f32 output

All K=64 remainder matmuls are zero-padded to K=128 (host-padded weights,
zero-filled y1p tail rows) so every LDWEIGHTS is a uniform 128-row load.
"""
import numpy as np
import ml_dtypes

import concourse.bass as bass
import concourse.tile as tile
from concourse import bacc, mybir
from concourse.bass_utils import run_bass_kernel_spmd

BF16NP = ml_dtypes.bfloat16
BF = mybir.dt.bfloat16
F32 = mybir.dt.float32

B, C, H, W = 8, 192, 128, 128
HEADS, CH = 4, 48
C3 = 3 * C                      # 576
C3P = 640                       # 576 padded to 5*128
CP2 = 256                       # 192 padded to 2*128
HP, WP = H + 2, W + 2           # 130
NPIX = H * W                    # 16384
NPP = HP * WP                   # 16900
NT = 512
NTB = NPIX // NT                # 32 conv n-tiles
NTA = 34                        # stage-A n-tiles (33x512 + 1 overlapping)
KT = [(0, 128), (128, 128), (256, 128), (384, 128), (512, 64)]   # 576 split
MT_OUT = [(0, 128), (128, 64)]                                   # out-ch split

_CACHE = {}


def _build(variant="full"):
    nc = bacc.Bacc("TRN2", target_bir_lowering=False, debug=False, num_devices=8)
    xp_d = nc.dram_tensor("xp", [CP2, NPP], BF, kind="ExternalInput").ap()
    w1t_d = nc.dram_tensor("w1t", [CP2, C3], BF, kind="ExternalInput").ap()
    w2sb_d = nc.dram_tensor("w2sb", [5, 128, 9 * C3], BF, kind="ExternalInput").ap()
    wpt_d = nc.dram_tensor("wpt", [CH, HEADS * C], BF, kind="ExternalInput").ap()
    scale_d = nc.dram_tensor("scale", [1, HEADS], F32, kind="ExternalInput").ap()
    out_d = nc.dram_tensor("out", [C, NPIX], F32, kind="ExternalOutput").ap()

    with tile.TileContext(nc) as tc:
        with tc.tile_pool(name="dram", bufs=1, space="DRAM") as dram:
            y1p = dram.tile([C3P, NPP], BF)
            qk = dram.tile([2 * C, NPIX], BF)
            rinv_d = dram.tile([1, 512], F32)
            fwt_d = dram.tile([C, C], BF)
            _build_body(nc, tc, xp_d, w1t_d, w2sb_d, wpt_d, scale_d, out_d,
                        y1p, qk, rinv_d, fwt_d, variant)
    nc.compile()
    return nc


def _build_body(nc, tc, xp_d, w1t_d, w2sb_d, wpt_d, scale_d, out_d,
                y1p, qk, rinv_d, fwt_d, variant="full"):
    X = mybir.AxisListType.X

    with tc.tile_pool(name="persist", bufs=1) as persist:
        v0 = persist.tile([128, NPIX], BF, tag="v0")
        v1 = persist.tile([64, NPIX], BF, tag="v1")
        parts = [persist.tile([mp, NTB], F32, tag=f"part{i}", name=f"part{i}")
                 for i, (m0, mp) in enumerate(KT[:3])]
        psG_ctx = tc.tile_pool(name="psG", bufs=1, space="PSUM")
        psG = psG_ctx.__enter__()
        Gall = None

        # ---------------- Phase A + B (+ interleaved logits) ----------------
        with (tc.tile_pool(name="wts", bufs=1) as wts,
              tc.tile_pool(name="xk", bufs=6) as xkp,
              tc.tile_pool(name="slab", bufs=10) as slabp,
              tc.tile_pool(name="stage", bufs=8) as stagep,
              tc.tile_pool(name="sq", bufs=3) as sqp,
              tc.tile_pool(name="qkt", bufs=8) as qktp,
              tc.tile_pool(name="psA", bufs=6, space="PSUM") as psA):

            w1s = []
            for i in range(2):
                t = wts.tile([128, C3], BF, tag=f"w1_{i}", name=f"w1_{i}")
                nc.sync.dma_start(t[:], w1t_d[128 * i:128 * (i + 1), :])
                w1s.append(t)
            w2s = []
            for i in range(5):
                t = wts.tile([128, 9 * C3], BF, tag=f"w2_{i}", name=f"w2_{i}")
                nc.sync.dma_start(t[:], w2sb_d[i, :, :])
                w2s.append(t)

            # zero-fill y1p rows 576..640 once (K=128 padding for conv kt=4)
            zst = wts.tile([64, 2048], BF, tag="zst")
            nc.vector.memset(zst[:], 0.0)
            for a in range(NPP // 2048 + 1):
                o = a * 2048 if a < NPP // 2048 else NPP - 2048
                nc.sync.dma_start(y1p[C3:C3P, o:o + 2048], zst[:])

            # Phase A: y1p = w1 @ xp
            for t in range(NTA):
                off = t * NT if t < NTA - 1 else NPP - NT
                xks = []
                for i in range(2):
                    xk = xkp.tile([128, NT], BF, tag="xk")
                    nc.sync.dma_start(xk[:], xp_d[128 * i:128 * (i + 1), off:off + NT])
                    xks.append(xk)
                for mi, (m0, mp) in enumerate(KT):
                    ps = psA.tile([128, NT], F32, tag="ps")
                    for i in range(2):
                        nc.tensor.matmul(ps[:mp], w1s[i][:, m0:m0 + mp],
                                         xks[i][:], start=(i == 0), stop=(i == 1))
                    st = stagep.tile([128, NT], BF, tag="stage")
                    if mi % 2 == 0:
                        nc.vector.tensor_copy(st[:mp], ps[:mp])
                    else:
                        nc.scalar.copy(st[:mp], ps[:mp])
                    nc.sync.dma_start(y1p[m0:m0 + mp, off:off + NT], st[:mp])

            # logits PSUM accumulator: 4 heads packed in one bank [48, 192]
            Gall = psG.tile([CH, HEADS * CH], F32, tag="Gall")

            def issue_logits(tt):
                """4 chunk-transposes for conv n-tile tt were issued after
                tile tt's groups; the G matmuls for tile tt are issued here
                (one tile later) so the transpose DMA has a full tile of
                compute to hide under."""
                for j in range(4 * tt, 4 * tt + 4):
                    qkt = qktp.tile([128, 2 * C], BF, tag="qkt", name="qkt")
                    nc.sync.dma_start_transpose(qkt[:], qk[:, j * 128:(j + 1) * 128])
                    for h in range(HEADS):
                        nc.tensor.matmul(
                            Gall[:, CH * h:CH * (h + 1)],
                            qkt[:, CH * h:CH * (h + 1)],
                            qkt[:, C + CH * h:C + CH * (h + 1)],
                            start=(j == 0 and h == 0),
                            stop=(j == 127 and h == HEADS - 1),
                            skip_group_check=True)

            # Phase B: conv3x3 via 9 shifted matmuls (+ lagged logits)
            y1p_img = y1p.rearrange("c (h w) -> c h w", h=HP)
            for t in range(NTB):
                slabs = []
                for i, (k0, kp) in enumerate(KT):
                    sl = slabp.tile([128, 6, WP], BF, tag="slab")
                    nc.sync.dma_start(sl[:], y1p_img[128 * i:128 * (i + 1),
                                                     4 * t:4 * t + 6, :])
                    slabs.append(sl)
                for mi, (m0, mp) in enumerate(KT):
                    ps = psA.tile([128, NT], F32, tag="ps")
                    n_mm = 0
                    for s in range(9):
                        dy, dx = s // 3, s % 3
                        for i in range(5):
                            nc.tensor.matmul(
                                ps[:mp],
                                w2s[i][:, s * C3 + m0: s * C3 + m0 + mp],
                                slabs[i][:, dy:dy + 4, dx:dx + W],
                                start=(n_mm == 0), stop=(n_mm == 44))
                            n_mm += 1
                    if mi >= 3:   # v channels -> SBUF resident
                        vt, vp = (v0, 128) if mi == 3 else (v1, 64)
                        if mi == 3:
                            nc.scalar.copy(vt[:vp, t * NT:(t + 1) * NT], ps[:vp])
                        else:
                            nc.vector.tensor_copy(vt[:vp, t * NT:(t + 1) * NT], ps[:vp])
                        if variant == "ab":
                            stf = stagep.tile([128, NT], F32, tag="stagef",
                                              name="stf")
                            nc.any.tensor_copy(stf[:mp], ps[:mp])
                            nc.sync.dma_start(
                                out_d[m0 - 384:m0 - 384 + mp, t * NT:(t + 1) * NT],
                                stf[:mp])
                    else:         # q/k channels -> DRAM + norm partials
                        st = stagep.tile([128, NT], BF, tag="stage")
                        if mi % 2 == 0:
                            nc.vector.tensor_copy(st[:mp], ps[:mp])
                        else:
                            nc.scalar.copy(st[:mp], ps[:mp])
                        sq = sqp.tile([128, NT], F32, tag="sq")
                        nc.vector.tensor_mul(sq[:mp], st[:mp], st[:mp])
                        nc.vector.reduce_sum(parts[mi][:mp, t:t + 1], sq[:mp], axis=X)
                        nc.sync.dma_start(qk[m0:m0 + mp, t * NT:(t + 1) * NT], st[:mp])
                if variant != "ab":
                    # transposes for tile t (DMA, hides under tile t+1 compute)
                    for j in range(4 * t, 4 * t + 4):
                        pass  # issued inside issue_logits with 1-tile lag
                    if t >= 1:
                        issue_logits(t - 1)
            if variant == "ab":
                return
            issue_logits(NTB - 1)

        # -------- Phase C (small): norms, softmax, FW --------
        with (tc.tile_pool(name="small", bufs=1) as smallp,
              tc.tile_pool(name="soft", bufs=2) as softp,
              tc.tile_pool(name="psF", bufs=1, space="PSUM") as psF):
            for mi, (m0, mp) in enumerate(KT[:3]):
                ssq = smallp.tile([128, 1], F32, tag=f"ssq{mi}", name=f"ssq{mi}")
                nc.vector.reduce_sum(ssq[:mp], parts[mi][:mp, :], axis=X)
                nc.scalar.sqrt(ssq[:mp], ssq[:mp])
                nc.vector.reciprocal(ssq[:mp], ssq[:mp])
                nc.sync.dma_start(rinv_d[0, 128 * mi:128 * mi + mp], ssq[:mp, 0])

            alpha = smallp.tile([CH, HEADS], F32, tag="alpha")
            for h in range(HEADS):
                nc.sync.dma_start(alpha[:, h:h + 1], rinv_d[0, CH * h:CH * (h + 1)])
            scs = smallp.tile([CH, HEADS], F32, tag="scs")
            nc.gpsimd.dma_start(out=scs[:],
                                in_=scale_d[0:1, :].to_broadcast((CH, HEADS)))
            nc.vector.tensor_mul(alpha[:], alpha[:], scs[:])
            ball = smallp.tile([CH, C], F32, tag="ball")
            nc.gpsimd.dma_start(out=ball[:],
                                in_=rinv_d[0:1, C:2 * C].to_broadcast((CH, C)))

            wpt_sb = smallp.tile([CH, HEADS * C], BF, tag="wpt")
            nc.sync.dma_start(wpt_sb[:], wpt_d[:, :])

            for h in range(HEADS):
                gh = Gall[:, CH * h:CH * (h + 1)]
                nc.vector.tensor_scalar_mul(gh, gh, alpha[:, h:h + 1])
                gsb = softp.tile([CH, CH], F32, tag="gsb")
                nc.vector.tensor_mul(gsb[:], gh, ball[:, CH * h:CH * (h + 1)])
                mx = softp.tile([CH, 1], F32, tag="mx")
                nc.vector.reduce_max(mx[:], gsb[:], axis=X)
                nc.vector.tensor_scalar_mul(mx[:], mx[:], -1.0)
                ex = softp.tile([CH, CH], F32, tag="ex")
                nc.scalar.activation(ex[:], gsb[:],
                                     mybir.ActivationFunctionType.Exp,
                                     bias=mx[:], scale=1.0)
                sm = softp.tile([CH, 1], F32, tag="sm")
                nc.vector.reduce_sum(sm[:], ex[:], axis=X)
                nc.vector.reciprocal(sm[:], sm[:])
                asb = softp.tile([CH, CH], BF, tag="asb")
                nc.vector.tensor_scalar_mul(asb[:], ex[:], sm[:, 0:1])
                fw_ps = psF.tile([CH, C], F32, tag="fw")
                nc.tensor.matmul(fw_ps[:], asb[:], wpt_sb[:, C * h:C * (h + 1)],
                                 start=True, stop=True)
                fw_sb = softp.tile([CH, C], BF, tag="fwsb")
                nc.any.tensor_copy(fw_sb[:], fw_ps[:])
                nc.sync.dma_start(fwt_d[CH * h:CH * (h + 1), :], fw_sb[:])

            fwt0 = smallp.tile([128, C], BF, tag="fwt0")
            nc.sync.dma_start(fwt0[:], fwt_d[0:128, :])
            fwt1 = smallp.tile([64, C], BF, tag="fwt1")
            nc.sync.dma_start(fwt1[:], fwt_d[128:C, :])

            # -------- Phase D: out = FW @ v --------
            with (tc.tile_pool(name="ostage", bufs=6) as ostagep,
                  tc.tile_pool(name="psD", bufs=4, space="PSUM") as psD):
                for t in range(NTB):
                    for oi, (m0, mp) in enumerate(MT_OUT):
                        ps = psD.tile([128, NT], F32, tag="psD")
                        nc.tensor.matmul(ps[:mp], fwt0[:, m0:m0 + mp],
                                         v0[:, t * NT:(t + 1) * NT],
                                         start=True, stop=False)
                        nc.tensor.matmul(ps[:mp], fwt1[:, m0:m0 + mp],
                                         v1[:, t * NT:(t + 1) * NT],
                                         start=False, stop=True)
                        ost = ostagep.tile([128, NT], F32, tag="ost")
                        if (t + oi) % 2 == 0:
                            nc.vector.tensor_copy(ost[:mp], ps[:mp])
                        else:
                            nc.scalar.copy(ost[:mp], ps[:mp])
                        nc.sync.dma_start(out_d[m0:m0 + mp, t * NT:(t + 1) * NT],
                                          ost[:mp])
        psG_ctx.__exit__(None, None, None)


def _prep_shared(w_qkv1, w_qkv2, w_proj, scale):
    w1t = np.zeros((CP2, C3), dtype=BF16NP)
    w1t[:C] = np.ascontiguousarray(w_qkv1[:, :, 0, 0].T).astype(BF16NP)
    w2t = np.transpose(w_qkv2, (2, 3, 1, 0)).reshape(9, C3, C3)          # [s,i,o]
    w2sb = np.zeros((5, 128, 9 * C3), dtype=BF16NP)
    for kt, (k0, kp) in enumerate(KT):
        w2sb[kt, :kp, :] = np.ascontiguousarray(
            np.transpose(w2t[:, k0:k0 + kp, :], (1, 0, 2)).reshape(kp, 9 * C3)
        ).astype(BF16NP)
    wpf = w_proj[:, :, 0, 0].T                                            # [c,o]
    wpt = np.concatenate([wpf[h * CH:(h + 1) * CH, :] for h in range(HEADS)],
                         axis=1).astype(BF16NP)                           # [48,768]
    sc = np.asarray(scale, np.float32).reshape(1, HEADS)
    return w1t, w2sb, wpt, sc


def kernel(x, w_qkv1, w_qkv2, w_proj, scale):
    x = np.asarray(x, np.float32)
    if "nc" not in _CACHE:
        _CACHE["nc"] = _build()
    nc = _CACHE["nc"]

    w1t, w2sb, wpt, sc = _prep_shared(
        np.asarray(w_qkv1, np.float32), np.asarray(w_qkv2, np.float32),
        np.asarray(w_proj, np.float32), np.asarray(scale, np.float32))

    xp = np.zeros((B, CP2, HP, WP), np.float32)
    xp[:, :C, 1:H + 1, 1:W + 1] = x
    xp = xp.astype(BF16NP).reshape(B, CP2, NPP)

    in_maps = [{"xp": xp[i], "w1t": w1t, "w2sb": w2sb, "wpt": wpt, "scale": sc}
               for i in range(B)]
    res = run_bass_kernel_spmd(nc, in_maps, core_ids=list(range(B)))
    out = np.stack([res.results[i]["out"].reshape(C, H, W) for i in range(B)], 0)
    return np.ascontiguousarray(out.astype(np.float32))

